# revision 7
# baseline (speedup 1.0000x reference)
"""Trainium2 Bass kernel for EnhancedMotionFlow (fp8 DoubleRow version).

Pure data-parallel: batch dim B=256 sharded 32-per-core across 8 cores;
tiny weights replicated. No collectives.

Math (per token t = (b, T, J)):
  orig   = relu(x @ ow1 + ob1) @ ow2 + ob2                    x: 3 -> 64
  for each scale s in (1, 8, 25, 100):
    m_s  = depthwise temporal conv of x (clamped pad)         taps in {t-2..t+2}
    f_s  = relu(relu(m_s @ e_w1 + b1) @ e_w2 + b2) * softmax(sw)[s]
  combined = concat(f_s)                                      -> 64
  tmot  = relu(relu(combined @ tw1 + tb1) @ tw2 + tb2)        64 -> 128 -> 64
  out   = orig + tmot

Device-side structure (feature-major, tokens on the free axis, fp8 matmuls
in DoubleRow perf mode = 2 columns/cycle):
  - Host prebuilds xs8 [24, NTOK] fp8: row 0 = ones (bias row), rows 1..15 =
    (tap, channel) pre-shifted + edge-clamped token streams; rows 16:20 =
    fp8 of the fp32-vs-fp8 residual of rows 0:4; rows 20:24 = copy of rows
    0:4. The residual rows + residual weights make the orig-embed L1 run at
    ~fp16 accuracy out of an fp8 matmul (error ~ eps^2).
  - Each chunk-pair (2 x 512 tokens) is consumed via [p, 2, 512] DoubleRow
    access patterns; zero-weighted subtile halves select chunk i or j.
  - conv+encoder-L1 of all 4 branches fused into one K=16 matmul (w1c).
  - encoder L2: one K=256 DoubleRow matmul over both chunks' h1.
  - transform L1: broadcast (stride-0) rhs DoubleRow over comb halves.
  - final: engines can read only ONE PSUM operand per instruction, so
    orig (F) and tmot (E) are evicted to SBUF bf16 on Act/DVE and the
    final add runs on the Pool engine (SBUF-only), which is otherwise idle.
  - eviction work is column-split between Act and DVE to balance busy time.
  - output stored feature-major bf16, one DMA per 2 pairs; the host
    reassembles token order, upcasts, and adds ob2.
"""

import sys
import functools

import numpy as np
import ml_dtypes

sys.path.insert(0, "/opt/trn_rl_repo")

from contextlib import ExitStack  # noqa: E402

import concourse.bacc as bacc  # noqa: E402
import concourse.tile as tile  # noqa: E402
from concourse import mybir  # noqa: E402
from concourse.bass_utils import run_bass_kernel_spmd  # noqa: E402

F32 = mybir.dt.float32
BF16 = mybir.dt.bfloat16
F8 = mybir.dt.float8e4
DR = mybir.MatmulPerfMode.DoubleRow
Relu = mybir.ActivationFunctionType.Relu
ADD = mybir.AluOpType.add
MAX = mybir.AluOpType.max
NP8 = ml_dtypes.float8_e4m3fn
NPB = ml_dtypes.bfloat16

B, T, J, C = 256, 243, 17, 3
D = 64
SD = 16
SCALES = (1, 8, 25, 100)
NCORES = 8
BLOC = B // NCORES            # batches per core
NTOK_FULL = BLOC * T * J      # tokens per core = 132192

CH = 512                      # tokens per chunk = one full PSUM bank of fp32
PAIR = 2 * CH                 # tokens per chunk-pair = 1024
NTOK_PAD = -(-NTOK_FULL // (4 * PAIR)) * (4 * PAIR)  # 135168 (33 x 4 pairs)

TAP_ORDER = (0, -2, -1, +1, +2)


def q8(a):
    return np.asarray(a, np.float32).astype(NP8)


# ---------------------------------------------------------------------------
# host-side preprocessing
# ---------------------------------------------------------------------------

def _host_prep_x(x_shard: np.ndarray, ntok: int) -> np.ndarray:
    """x_shard [BLOC', T, J, C] -> xs8 [24, ntok] fp8 (e4m3).

    Rows 0:16: row 0 all-ones; row 1 + 3*g + c is channel c shifted by
    TAP_ORDER[g] t-steps with clamped padding. Rows 16:20: fp8 residual of
    rows 0:4 (fp32 value minus its fp8 rounding). Rows 20:24: rows 0:4 again.
    Token order: tok = b*T*J + t*J + j.
    """
    bl = x_shard.shape[0]
    full = bl * T * J
    n = min(full, ntok)
    xs = np.zeros((16, ntok), dtype=np.float32)
    xs[0, :] = 1.0
    tidx = np.arange(T)
    for g, off in enumerate(TAP_ORDER):
        src_t = np.clip(tidx + off, 0, T - 1)
        shifted = x_shard[:, src_t, :, :]             # [bl, T, J, C]
        for c in range(C):
            xs[1 + 3 * g + c, :n] = shifted[:, :, :, c].reshape(full)[:n]
    out = np.zeros((24, ntok), dtype=NP8)
    out[0:16] = xs.astype(NP8)
    top4 = xs[0:4]
    out[16:20] = (top4 - out[0:4].astype(np.float32)).astype(NP8)
    out[20:24] = out[0:4]
    return out


def _host_prep_weights(p: dict) -> dict:
    """Fold the tiny parameters into device weight tiles."""
    w = {}
    sw = np.asarray(p["sw"], np.float32)
    e = np.exp(sw - sw.max())
    wts = (e / e.sum()).astype(np.float32)

    # w1c [16, 128]: fused conv taps x encoder L1, L1 bias in the ones row.
    w1c = np.zeros((16, 128), np.float32)
    for i, s in enumerate(SCALES):
        ks = min(s + 1, 5)
        kern = np.asarray(p[f"k{s}"], np.float32)      # [ks, C]
        w1 = np.asarray(p[f"e{s}w1"], np.float32)      # [C, 32]
        b1 = np.asarray(p[f"e{s}b1"], np.float32)      # [32]
        for k in range(ks):
            off = k - ks // 2
            g = TAP_ORDER.index(off)
            for c in range(C):
                w1c[1 + 3 * g + c, 32 * i:32 * i + 32] += kern[k, c] * w1[c, :]
        w1c[0, 32 * i:32 * i + 32] = b1
    w1c8 = q8(w1c)
    # DoubleRow lhsT [24, 2, 128] stored [24, 256]: s-major blocks of 128.
    awi = np.zeros((24, 256), NP8)
    awi[0:16, 0:128] = w1c8
    awj = np.zeros((24, 256), NP8)
    awj[0:16, 128:256] = w1c8
    w["awi"], w["awj"] = awi, awj

    # orig embed L1 with fp8 residual compensation. o1 [4, 64] (ones row =
    # ob1). lhsT rows 0:4 -> o18 (@x8), 16:20 -> o18 (@resid), 20:24 -> o1r8
    # (@x8 copy). s0 drives out cols 0:64 (chunk i), s1 cols 64:128.
    o1 = np.zeros((4, D), np.float32)
    o1[0, :] = np.asarray(p["ob1"], np.float32)
    o1[1:4, :] = np.asarray(p["ow1"], np.float32)
    o18 = q8(o1)
    o1r8 = q8(o1 - o18.astype(np.float32))
    bw = np.zeros((24, 256), NP8)
    for s_blk, col0 in ((0, 0), (1, 64)):
        base = 128 * s_blk
        bw[0:4, base + col0:base + col0 + 64] = o18
        bw[16:20, base + col0:base + col0 + 64] = o18
        bw[20:24, base + col0:base + col0 + 64] = o1r8
    w["bw"] = bw

    # encoder L2 block-diagonal [128, 64] with softmax weights folded.
    w2 = np.zeros((128, D), np.float32)
    b2c = np.zeros((D,), np.float32)
    for i, s in enumerate(SCALES):
        w2[32 * i:32 * i + 32, 16 * i:16 * i + 16] = wts[i] * np.asarray(
            p[f"e{s}w2"], np.float32)
        b2c[16 * i:16 * i + 16] = wts[i] * np.asarray(p[f"e{s}b2"], np.float32)
    w28 = q8(w2)
    cw = np.zeros((128, 256), NP8)
    cw[:, 0:64] = w28          # s0 -> comb_i (out cols 0:64)
    cw[:, 128 + 64:128 + 128] = w28  # s1 -> comb_j (out cols 64:128)
    w["cw"] = cw
    w["b2x2"] = np.concatenate([b2c, b2c]).reshape(128, 1).astype(np.float32)

    # transform L1 [64, 128] fp8; broadcast-rhs DoubleRow, s1 weights zero.
    # dw rows 0:64 serve D_i (comb partitions 0:64); rows 64:128 serve D_j.
    tw18 = q8(np.asarray(p["tw1"], np.float32))        # [64, 128]
    dw = np.zeros((128, 256), NP8)
    dw[0:64, 0:128] = tw18
    dw[64:128, 0:128] = tw18
    w["dw"] = dw
    w["tb1"] = np.asarray(p["tb1"], np.float32).reshape(128, 1)

    # transform L2 DoubleRow lhsT [128, 2, 128]: s0 -> out cols 0:64 (tmot_i),
    # s1 -> cols 64:128 (tmot_j).
    tw28 = q8(np.asarray(p["tw2"], np.float32))        # [128, 64]
    ew = np.zeros((128, 256), NP8)
    ew[:, 0:64] = tw28
    ew[:, 128 + 64:128 + 128] = tw28
    w["ew"] = ew
    tb2 = np.asarray(p["tb2"], np.float32)
    w["tb2x2"] = np.concatenate([tb2, tb2]).reshape(128, 1).astype(np.float32)

    # orig embed L2 block-diag bf16 (precision-critical path).
    ow2 = np.asarray(p["ow2"], np.float32)             # [64, 64]
    o2bd = np.zeros((128, 128), np.float32)
    o2bd[:64, :64] = ow2
    o2bd[64:, 64:] = ow2
    w["fw"] = o2bd.astype(NPB)
    return w


# name -> (shape, dtype kind)
WEIGHT_SHAPES = {
    "awi": ((24, 256), "8"), "awj": ((24, 256), "8"), "bw": ((24, 256), "8"),
    "cw": ((128, 256), "8"), "b2x2": ((128, 1), "f"),
    "dw": ((128, 256), "8"), "tb1": ((128, 1), "f"),
    "ew": ((128, 256), "8"), "tb2x2": ((128, 1), "f"),
    "fw": ((128, 128), "b"),
}
_DT = {"8": F8, "b": BF16, "f": F32}
_NPDT = {"8": NP8, "b": NPB, "f": np.float32}


# ---------------------------------------------------------------------------
# device kernel
# ---------------------------------------------------------------------------

# column split of the hop eviction: first HOP_ACT cols on Act, rest on DVE
HOP_ACT = 192


def _emit(ctx: ExitStack, tc: tile.TileContext, ntok: int,
          xs_d, w_d, out_d, zero_tb2: bool = True, repeat: int = 1):
    nc = tc.nc
    npairs = ntok // PAIR
    assert npairs * PAIR == ntok and npairs % 4 == 0

    singles = ctx.enter_context(tc.tile_pool(name="singles", bufs=1))
    xpool = ctx.enter_context(tc.tile_pool(name="xin", bufs=2))
    work = ctx.enter_context(tc.tile_pool(name="work", bufs=2))
    outp = ctx.enter_context(tc.tile_pool(name="outs", bufs=2))
    psum = ctx.enter_context(tc.tile_pool(name="psum", bufs=1, space="PSUM"))

    wt = {}
    for name, (shape, kind) in WEIGHT_SHAPES.items():
        t = singles.tile(list(shape), _DT[kind], tag=f"w_{name}")
        nc.sync.dma_start(out=t[:], in_=w_d[name][:])
        wt[name] = t

    def dr(ap2d, s=2):
        return ap2d.rearrange("p (s n) -> p s n", s=s)

    awi = dr(wt["awi"][:])
    awj = dr(wt["awj"][:])
    bw = dr(wt["bw"][:])
    cw = dr(wt["cw"][:])
    dwi = dr(wt["dw"][0:64, :])
    dwj = dr(wt["dw"][64:128, :])
    ew = dr(wt["ew"][:])

    # x ring: 2 slots of 4 pairs each
    xring = singles.tile([24, 2 * 4 * PAIR], F8, tag="xring")

    for pi_r in range(npairs * repeat):
        pi = pi_r % npairs
        slot = (pi // 4) % 2
        if pi % 4 == 0:
            nc.sync.dma_start(
                out=xring[:, slot * 4 * PAIR:(slot + 1) * 4 * PAIR],
                in_=xs_d[:, pi * PAIR:(pi + 4) * PAIR])
        xo = slot * 4 * PAIR + (pi % 4) * PAIR
        rhs24 = dr(xring[:, xo:xo + PAIR])

        # ---- branch L1 (conv + encoder L1 fused): h1 = relu(w1c^T x)
        A = psum.tile([128, 1024], F32, tag="A")
        nc.tensor.matmul(A[:, 0:512], awi, rhs24, start=True, stop=True,
                         perf_mode=DR)
        nc.tensor.matmul(A[:, 512:1024], awj, rhs24, start=True, stop=True,
                         perf_mode=DR)
        h1 = work.tile([128, 1024], F8, tag="h1", bufs=3)
        nc.scalar.activation(h1[:], A[:], Relu)

        # ---- orig embed L1 (residual-compensated): hop = relu(o1^T x)
        Bp = psum.tile([128, 512], F32, tag="B")
        nc.tensor.matmul(Bp[:], bw, rhs24, start=True, stop=True, perf_mode=DR)
        hop = work.tile([128, 512], BF16, tag="hop", bufs=3)
        nc.scalar.activation(hop[:, 0:HOP_ACT], Bp[:, 0:HOP_ACT], Relu)
        nc.vector.tensor_scalar(hop[:, HOP_ACT:512], Bp[:, HOP_ACT:512],
                                0.0, None, op0=MAX)

        # ---- encoder L2, K=256 DoubleRow over both chunks' h1
        Cp = psum.tile([128, 512], F32, tag="C")
        nc.tensor.matmul(Cp[:], cw, dr(h1[:]), start=True, stop=True,
                         perf_mode=DR)
        comb = work.tile([128, 512], F8, tag="comb", bufs=3)
        nc.vector.tensor_scalar(comb[:], Cp[:], wt["b2x2"][:, 0:1], 0.0,
                                op0=ADD, op1=MAX)

        # ---- transform L1: h2 = relu(tw1^T comb + tb1), broadcast-rhs DR
        Dp = psum.tile([128, 1024], F32, tag="D")
        nc.tensor.matmul(
            Dp[:, 0:512], dwi,
            comb[0:64, :].unsqueeze(1).broadcast_to([64, 2, 512]),
            start=True, stop=True, perf_mode=DR)
        nc.tensor.matmul(
            Dp[:, 512:1024], dwj,
            comb[64:128, :].unsqueeze(1).broadcast_to([64, 2, 512]),
            start=True, stop=True, perf_mode=DR)
        h2 = work.tile([128, 1024], F8, tag="h2", bufs=3)
        nc.scalar.activation(h2[:], Dp[:], Relu, bias=wt["tb1"][:, 0:1])

        # ---- transform L2: tmot = relu(tw2^T h2 + tb2), K=256 DR
        Ep = psum.tile([128, 512], F32, tag="E")
        nc.tensor.matmul(Ep[:], ew, dr(h2[:]), start=True, stop=True,
                         perf_mode=DR)
        tmot = work.tile([128, 512], BF16, tag="tmot", bufs=3)
        nc.vector.tensor_scalar(tmot[:], Ep[:], wt["tb2x2"][:, 0:1], 0.0,
                                op0=ADD, op1=MAX)

        # ---- orig embed L2: forig = o2bd^T hop (bf16, no relu)
        Fp = psum.tile([128, 512], F32, tag="F")
        nc.tensor.matmul(Fp[:], wt["fw"][:], hop[:], start=True, stop=True)
        forig = work.tile([128, 512], BF16, tag="forig", bufs=3)
        nc.vector.tensor_scalar(forig[:], Fp[:], 0.0, None, op0=ADD)

        # ---- final add on Pool (SBUF-only engine): fm = forig + tmot
        if pi % 2 == 0:
            fm2 = outp.tile([128, 2 * 512], BF16, tag="fm2", bufs=2)
        fslice = fm2[:, (pi % 2) * 512:(pi % 2) * 512 + 512]
        nc.gpsimd.tensor_tensor(fslice, forig[:], tmot[:], op=ADD)

        # ---- store 2 pairs per DMA, feature-major bf16
        if pi % 2 == 1:
            base = (pi - 1) * 128
            nc.sync.dma_start(
                out=out_d[base:base + 256, :].rearrange("(s p) n -> p s n",
                                                        p=128),
                in_=dr(fm2[:]))


@functools.lru_cache(maxsize=8)
def _build_nc(ntok: int, repeat: int = 1, zero_tb2: bool = True):
    nc = bacc.Bacc("TRN2", target_bir_lowering=False, debug=False)
    xs_d = nc.dram_tensor("xs8", [24, ntok], F8, kind="ExternalInput").ap()
    w_d = {}
    for name, (shape, kind) in WEIGHT_SHAPES.items():
        w_d[name] = nc.dram_tensor(name, list(shape), _DT[kind],
                                   kind="ExternalInput").ap()
    npairs = ntok // PAIR
    out_d = nc.dram_tensor("out", [npairs * 128, 512], BF16,
                           kind="ExternalOutput").ap()
    with tile.TileContext(nc) as tc:
        with ExitStack() as ctx:
            _emit(ctx, tc, ntok, xs_d, w_d, out_d, zero_tb2=zero_tb2,
                  repeat=repeat)
    nc.compile()
    return nc


# ---------------------------------------------------------------------------
# entry point
# ---------------------------------------------------------------------------

LAST_RESULT = None


def _unshard_out(raw: np.ndarray) -> np.ndarray:
    """[npairs*128, 512] bf16 -> [BLOC, T, J, D] f32 token-major."""
    npairs = NTOK_PAD // PAIR
    a = raw.astype(np.float32).reshape(npairs, 2, D, CH)  # [p, ch, feat, n]
    a = a.transpose(0, 1, 3, 2).reshape(npairs * PAIR, D)
    return a[:NTOK_FULL].reshape(BLOC, T, J, D)


def kernel(**inputs) -> np.ndarray:
    x = np.asarray(inputs["x"], np.float32)
    assert x.shape == (B, T, J, C)
    w = _host_prep_weights(inputs)
    ob2 = np.asarray(inputs["ob2"], np.float32)

    in_maps = []
    for ci in range(NCORES):
        shard = x[ci * BLOC:(ci + 1) * BLOC]
        m = {"xs8": _host_prep_x(shard, NTOK_PAD)}
        for name in WEIGHT_SHAPES:
            m[name] = w[name]
        in_maps.append(m)

    zt = not np.any(np.asarray(inputs["tb2"], np.float32))
    nc = _build_nc(NTOK_PAD, 1, zt)
    res = run_bass_kernel_spmd(nc, in_maps, list(range(NCORES)))
    global LAST_RESULT
    LAST_RESULT = res
    outs = [_unshard_out(res.results[i]["out"]) for i in range(NCORES)]
    full = np.concatenate(outs, axis=0)
    full = full + ob2.reshape(1, 1, 1, D)
    return full.astype(np.float32)


# revision 10
# speedup vs baseline: 1.7967x; 1.7967x over previous
"""Trainium2 Bass kernel for EnhancedMotionFlow (plain fp8 + f32r version).

Pure data-parallel: batch dim B=256 sharded 32-per-core across 8 cores;
tiny weights replicated. No collectives.

Math (per token t = (b, T, J)):
  orig   = relu(x @ ow1 + ob1) @ ow2 + ob2                    x: 3 -> 64
  for each scale s in (1, 8, 25, 100):
    m_s  = depthwise temporal conv of x (clamped pad)         taps in {t-2..t+2}
    f_s  = relu(relu(m_s @ e_w1 + b1) @ e_w2 + b2) * softmax(sw)[s]
  combined = concat(f_s)                                      -> 64
  tmot  = relu(relu(combined @ tw1 + tb1) @ tw2 + tb2)        64 -> 128 -> 64
  out   = orig + tmot

Measured-on-silicon design notes:
  - plain fp8 and f32r matmuls stream ~512 cols in ~160-200 ns with FREE
    weight switches (dual weight buffer); DoubleRow fp8 and bf16 pay ~480 ns
    per weight switch -> avoided entirely.
  - branch path (error-tolerant: averaging over K=64..256) runs fp8;
    the precision-critical orig path runs f32r end to end (exact).
  - engines can read only ONE PSUM operand per op and GPSIMD cannot read
    PSUM at all: orig (F) and tmot (E) are evicted to SBUF bf16 on Act/DVE
    and the final add runs on the otherwise-idle Pool engine.
  - eviction work is column-split between Act and DVE to balance busy time.
  - output stored feature-major bf16, one DMA per 2 pairs; the host
    reassembles token order, upcasts, and adds ob2.

Device-side structure (tokens on the free axis, chunk = 512 tokens,
pair = 2 chunks; per pair 11 matmuls / 7 eviction ops):
  xs8 [16, NTOK] fp8: row 0 = ones (bias row), rows 1..15 = (tap, channel)
    pre-shifted + edge-clamped token streams (conv folded into L1 weights).
  xf8 [8, NTOK/2] f32r: pair-folded ones + center-tap channels (chunk i
    rows 0:4, chunk j rows 4:8) for the exact f32r orig path.
  A_i/A_j: h1 = relu(w1c^T x)            [128, 512] x2   (fp8, K=16)
  B:       hop = relu(o1^T x) packed     [128, 512]      (f32r, K=8)
  C1+C2:   comb = relu(w2^T h1 + b2c)    [128, 512]      (fp8, K=128 x2 acc)
  D_i/D_j: h2 = relu(tw1^T comb + tb1)   [128, 512] x2   (fp8, K=128)
  E1+E2:   tmot = relu(tw2^T h2 + tb2)   [128, 512]      (fp8, K=128 x2 acc)
  F:       forig = o2bd^T hop            [128, 512]      (f32r, K=128)
  fm = forig + tmot on Pool -> bf16 -> HBM.
"""

import sys
import functools

import numpy as np
import ml_dtypes

sys.path.insert(0, "/opt/trn_rl_repo")

from contextlib import ExitStack  # noqa: E402

import concourse.bacc as bacc  # noqa: E402
import concourse.tile as tile  # noqa: E402
from concourse import mybir  # noqa: E402
from concourse.bass_utils import run_bass_kernel_spmd  # noqa: E402

F32 = mybir.dt.float32
F32R = mybir.dt.float32r
BF16 = mybir.dt.bfloat16
F8 = mybir.dt.float8e4
Relu = mybir.ActivationFunctionType.Relu
ADD = mybir.AluOpType.add
MAX = mybir.AluOpType.max
NP8 = ml_dtypes.float8_e4m3fn
NPB = ml_dtypes.bfloat16

B, T, J, C = 256, 243, 17, 3
D = 64
SD = 16
SCALES = (1, 8, 25, 100)
NCORES = 8
BLOC = B // NCORES            # batches per core
NTOK_FULL = BLOC * T * J      # tokens per core = 132192

CH = 512                      # tokens per chunk = one full PSUM bank of fp32
PAIR = 2 * CH                 # tokens per chunk-pair = 1024
NTOK_PAD = -(-NTOK_FULL // (4 * PAIR)) * (4 * PAIR)  # 135168 (33 x 4 pairs)

TAP_ORDER = (0, -2, -1, +1, +2)

# column split of the hop eviction: first HOP_ACT cols on Act, rest on DVE
HOP_ACT = 192


def q8(a):
    return np.asarray(a, np.float32).astype(NP8)


# ---------------------------------------------------------------------------
# host-side preprocessing
# ---------------------------------------------------------------------------

def _host_prep_x(x_shard: np.ndarray, ntok: int):
    """x_shard [BLOC', T, J, C] -> (xs8 [16, ntok] fp8, xf4 [4, ntok] f32).

    xs8: row 0 all-ones; row 1 + 3*g + c is channel c shifted by TAP_ORDER[g]
    t-steps with clamped padding. xf4 = rows 0:4 in f32 for the orig path.
    Token order: tok = b*T*J + t*J + j.
    """
    bl = x_shard.shape[0]
    full = bl * T * J
    n = min(full, ntok)
    xs = np.zeros((16, ntok), dtype=np.float32)
    xs[0, :] = 1.0
    tidx = np.arange(T)
    for g, off in enumerate(TAP_ORDER):
        src_t = np.clip(tidx + off, 0, T - 1)
        shifted = x_shard[:, src_t, :, :]             # [bl, T, J, C]
        for c in range(C):
            xs[1 + 3 * g + c, :n] = shifted[:, :, :, c].reshape(full)[:n]
    xf = xs[0:4]
    # fold pairs into K: xf8[0:4, pcol] = chunk i, xf8[4:8, pcol] = chunk j
    f = xf.reshape(4, ntok // PAIR, 2, CH)
    xf8 = np.concatenate([f[:, :, 0, :], f[:, :, 1, :]], axis=0)
    return xs.astype(NP8), np.ascontiguousarray(xf8.reshape(8, ntok // 2))


def _host_prep_weights(p: dict) -> dict:
    """Fold the tiny parameters into device weight tiles."""
    w = {}
    sw = np.asarray(p["sw"], np.float32)
    e = np.exp(sw - sw.max())
    wts = (e / e.sum()).astype(np.float32)

    # w1c [16, 128]: fused conv taps x encoder L1, L1 bias in the ones row.
    w1c = np.zeros((16, 128), np.float32)
    for i, s in enumerate(SCALES):
        ks = min(s + 1, 5)
        kern = np.asarray(p[f"k{s}"], np.float32)      # [ks, C]
        w1 = np.asarray(p[f"e{s}w1"], np.float32)      # [C, 32]
        b1 = np.asarray(p[f"e{s}b1"], np.float32)      # [32]
        for k in range(ks):
            off = k - ks // 2
            g = TAP_ORDER.index(off)
            for c in range(C):
                w1c[1 + 3 * g + c, 32 * i:32 * i + 32] += kern[k, c] * w1[c, :]
        w1c[0, 32 * i:32 * i + 32] = b1
    w["aw"] = q8(w1c)

    # orig embed L1 as one K=8 f32r matmul over the pair-folded xf8 stream:
    # rows 0:4 (chunk i) -> out cols 0:64, rows 4:8 (chunk j) -> cols 64:128.
    o1 = np.zeros((4, D), np.float32)
    o1[0, :] = np.asarray(p["ob1"], np.float32)
    o1[1:4, :] = np.asarray(p["ow1"], np.float32)
    bw = np.zeros((8, 128), np.float32)
    bw[0:4, 0:64] = o1
    bw[4:8, 64:128] = o1
    w["bw"] = bw

    # encoder L2 block-diagonal with softmax weights folded; split into the
    # two K=128 accumulation halves (cwa: h1_i -> comb cols 0:64 / cwb:
    # h1_j -> cols 64:128).
    w2 = np.zeros((128, D), np.float32)
    b2c = np.zeros((D,), np.float32)
    for i, s in enumerate(SCALES):
        w2[32 * i:32 * i + 32, 16 * i:16 * i + 16] = wts[i] * np.asarray(
            p[f"e{s}w2"], np.float32)
        b2c[16 * i:16 * i + 16] = wts[i] * np.asarray(p[f"e{s}b2"], np.float32)
    w28 = q8(w2).astype(np.float32)
    cwa = np.zeros((128, 128), np.float32)
    cwa[:, 0:64] = w28
    cwb = np.zeros((128, 128), np.float32)
    cwb[:, 64:128] = w28
    w["cwa"], w["cwb"] = cwa.astype(NP8), cwb.astype(NP8)
    w["b2x2"] = np.concatenate([b2c, b2c]).reshape(128, 1).astype(np.float32)

    # transform L1: dwa acts on comb rows 0:64 (comb_i), dwb on rows 64:128.
    tw18 = q8(np.asarray(p["tw1"], np.float32)).astype(np.float32)  # [64,128]
    dwa = np.zeros((128, 128), np.float32)
    dwa[0:64, :] = tw18
    dwb = np.zeros((128, 128), np.float32)
    dwb[64:128, :] = tw18
    w["dwa"], w["dwb"] = dwa.astype(NP8), dwb.astype(NP8)
    w["tb1"] = np.asarray(p["tb1"], np.float32).reshape(128, 1)

    # transform L2 accumulation halves: ewa: h2_i -> tmot cols 0:64, ewb:
    # h2_j -> cols 64:128.
    tw28 = q8(np.asarray(p["tw2"], np.float32)).astype(np.float32)  # [128,64]
    ewa = np.zeros((128, 128), np.float32)
    ewa[:, 0:64] = tw28
    ewb = np.zeros((128, 128), np.float32)
    ewb[:, 64:128] = tw28
    w["ewa"], w["ewb"] = ewa.astype(NP8), ewb.astype(NP8)
    tb2 = np.asarray(p["tb2"], np.float32)
    w["tb2x2"] = np.concatenate([tb2, tb2]).reshape(128, 1).astype(np.float32)

    # orig embed L2 block-diag f32 (f32r on device, exact).
    ow2 = np.asarray(p["ow2"], np.float32)             # [64, 64]
    o2bd = np.zeros((128, 128), np.float32)
    o2bd[:64, :64] = ow2
    o2bd[64:, 64:] = ow2
    w["fw"] = o2bd
    return w


# name -> (shape, dtype kind)
WEIGHT_SHAPES = {
    "aw": ((16, 128), "8"), "bw": ((8, 128), "r"),
    "cwa": ((128, 128), "8"), "cwb": ((128, 128), "8"),
    "b2x2": ((128, 1), "f"),
    "dwa": ((128, 128), "8"), "dwb": ((128, 128), "8"),
    "tb1": ((128, 1), "f"),
    "ewa": ((128, 128), "8"), "ewb": ((128, 128), "8"),
    "tb2x2": ((128, 1), "f"),
    "fw": ((128, 128), "r"),
}
_DT = {"8": F8, "b": BF16, "f": F32, "r": F32R}
_NPDT = {"8": NP8, "b": NPB, "f": np.float32, "r": np.float32}


# ---------------------------------------------------------------------------
# device kernel
# ---------------------------------------------------------------------------

def r(ap):
    return ap.bitcast(F32R)


def _emit(ctx: ExitStack, tc: tile.TileContext, ntok: int,
          xs_d, xf_d, w_d, out_d, repeat: int = 1):
    nc = tc.nc
    npairs = ntok // PAIR
    assert npairs * PAIR == ntok and npairs % 4 == 0

    singles = ctx.enter_context(tc.tile_pool(name="singles", bufs=1))
    work = ctx.enter_context(tc.tile_pool(name="work", bufs=2))
    outp = ctx.enter_context(tc.tile_pool(name="outs", bufs=2))
    psum = ctx.enter_context(tc.tile_pool(name="psum", bufs=1, space="PSUM"))

    wt = {}
    for name, (shape, kind) in WEIGHT_SHAPES.items():
        t = singles.tile(list(shape), _DT[kind], tag=f"w_{name}")
        nc.sync.dma_start(out=t[:], in_=w_d[name][:])
        wt[name] = t

    # x rings: 2 slots of 4 pairs each
    xring = singles.tile([16, 2 * 4 * PAIR], F8, tag="xring")
    fring = singles.tile([8, 2 * 4 * CH], F32R, tag="fring")

    for pi_r in range(npairs * repeat):
        pi = pi_r % npairs
        slot = (pi // 4) % 2
        if pi % 4 == 0:
            nc.sync.dma_start(
                out=xring[:, slot * 4 * PAIR:(slot + 1) * 4 * PAIR],
                in_=xs_d[:, pi * PAIR:(pi + 4) * PAIR])
            nc.sync.dma_start(
                out=fring[:, slot * 4 * CH:(slot + 1) * 4 * CH],
                in_=xf_d[:, pi * CH:(pi + 4) * CH])
        xo = slot * 4 * PAIR + (pi % 4) * PAIR
        x_i = xring[:, xo:xo + CH]
        x_j = xring[:, xo + CH:xo + PAIR]
        fo = slot * 4 * CH + (pi % 4) * CH
        f_p = fring[:, fo:fo + CH]

        # ---- branch L1 (conv + encoder L1 fused): h1 = relu(w1c^T x)
        A = psum.tile([128, 1024], F32, tag="A")
        nc.tensor.matmul(A[:, 0:512], wt["aw"][:], x_i, start=True, stop=True)
        nc.tensor.matmul(A[:, 512:1024], wt["aw"][:], x_j, start=True,
                         stop=True)
        h1 = work.tile([128, 1024], F8, tag="h1", bufs=3)
        nc.scalar.activation(h1[:], A[:], Relu)

        # ---- orig embed L1 (f32r, exact): hop = relu(o1^T x) packed,
        #      one K=8 matmul over the pair-folded stream
        Bp = psum.tile([128, 512], F32, tag="B")
        nc.tensor.matmul(Bp[:], wt["bw"][:], f_p, start=True, stop=True)
        hop = work.tile([128, 512], F32R, tag="hop", bufs=3)
        nc.scalar.activation(hop[:, 0:HOP_ACT], Bp[:, 0:HOP_ACT], Relu)
        nc.vector.tensor_scalar(hop[:, HOP_ACT:512], Bp[:, HOP_ACT:512],
                                0.0, None, op0=MAX)

        # ---- encoder L2: comb = relu(w2^T h1 + b2c), two-K accumulation
        Cp = psum.tile([128, 512], F32, tag="C")
        nc.tensor.matmul(Cp[:], wt["cwa"][:], h1[:, 0:512], start=True,
                         stop=False)
        nc.tensor.matmul(Cp[:], wt["cwb"][:], h1[:, 512:1024], start=False,
                         stop=True)
        comb = work.tile([128, 512], F8, tag="comb", bufs=3)
        nc.vector.tensor_scalar(comb[:], Cp[:], wt["b2x2"][:, 0:1], 0.0,
                                op0=ADD, op1=MAX)

        # ---- transform L1: h2 = relu(tw1^T comb + tb1)
        Dp = psum.tile([128, 1024], F32, tag="D")
        nc.tensor.matmul(Dp[:, 0:512], wt["dwa"][:], comb[:], start=True,
                         stop=True)
        nc.tensor.matmul(Dp[:, 512:1024], wt["dwb"][:], comb[:], start=True,
                         stop=True)
        h2 = work.tile([128, 1024], F8, tag="h2", bufs=3)
        nc.scalar.activation(h2[:], Dp[:], Relu, bias=wt["tb1"][:, 0:1])

        # ---- transform L2: tmot = relu(tw2^T h2 + tb2), two-K accumulation
        Ep = psum.tile([128, 512], F32, tag="E")
        nc.tensor.matmul(Ep[:], wt["ewa"][:], h2[:, 0:512], start=True,
                         stop=False)
        nc.tensor.matmul(Ep[:], wt["ewb"][:], h2[:, 512:1024], start=False,
                         stop=True)
        tmot = work.tile([128, 512], BF16, tag="tmot", bufs=3)
        nc.vector.tensor_scalar(tmot[:], Ep[:], wt["tb2x2"][:, 0:1], 0.0,
                                op0=ADD, op1=MAX)

        # ---- orig embed L2 (f32r, exact): forig = o2bd^T hop
        Fp = psum.tile([128, 512], F32, tag="F")
        nc.tensor.matmul(Fp[:], wt["fw"][:], hop[:], start=True,
                         stop=True)
        forig = work.tile([128, 512], BF16, tag="forig", bufs=3)
        nc.vector.tensor_scalar(forig[:], Fp[:], 0.0, None, op0=ADD)

        # ---- final add on Pool (SBUF-only engine): fm = forig + tmot
        if pi % 2 == 0:
            fm2 = outp.tile([128, 2 * 512], BF16, tag="fm2", bufs=2)
        fslice = fm2[:, (pi % 2) * 512:(pi % 2) * 512 + 512]
        nc.gpsimd.tensor_tensor(fslice, forig[:], tmot[:], op=ADD)

        # ---- store 2 pairs per DMA, feature-major bf16
        if pi % 2 == 1:
            base = (pi - 1) * 128
            nc.sync.dma_start(
                out=out_d[base:base + 256, :].rearrange("(s p) n -> p s n",
                                                        p=128),
                in_=fm2[:].rearrange("p (s n) -> p s n", s=2))


@functools.lru_cache(maxsize=8)
def _build_nc(ntok: int, repeat: int = 1):
    nc = bacc.Bacc("TRN2", target_bir_lowering=False, debug=False)
    xs_d = nc.dram_tensor("xs8", [16, ntok], F8, kind="ExternalInput").ap()
    xf_d = nc.dram_tensor("xf8", [8, ntok // 2], F32R,
                          kind="ExternalInput").ap()
    w_d = {}
    for name, (shape, kind) in WEIGHT_SHAPES.items():
        w_d[name] = nc.dram_tensor(name, list(shape), _DT[kind],
                                   kind="ExternalInput").ap()
    npairs = ntok // PAIR
    out_d = nc.dram_tensor("out", [npairs * 128, 512], BF16,
                           kind="ExternalOutput").ap()
    with tile.TileContext(nc) as tc:
        with ExitStack() as ctx:
            _emit(ctx, tc, ntok, xs_d, xf_d, w_d, out_d, repeat=repeat)
    nc.compile()
    return nc


# ---------------------------------------------------------------------------
# entry point
# ---------------------------------------------------------------------------

LAST_RESULT = None


def _unshard_out(raw: np.ndarray) -> np.ndarray:
    """[npairs*128, 512] bf16 -> [BLOC, T, J, D] f32 token-major."""
    npairs = NTOK_PAD // PAIR
    a = raw.astype(np.float32).reshape(npairs, 2, D, CH)  # [p, ch, feat, n]
    a = a.transpose(0, 1, 3, 2).reshape(npairs * PAIR, D)
    return a[:NTOK_FULL].reshape(BLOC, T, J, D)


def _make_in_maps(inputs) -> list:
    x = np.asarray(inputs["x"], np.float32)
    w = _host_prep_weights(inputs)
    in_maps = []
    for ci in range(NCORES):
        xs8, xf4 = _host_prep_x(x[ci * BLOC:(ci + 1) * BLOC], NTOK_PAD)
        m = {"xs8": xs8, "xf8": xf4}
        for name in WEIGHT_SHAPES:
            m[name] = w[name]
        in_maps.append(m)
    return in_maps


def kernel(**inputs) -> np.ndarray:
    x = np.asarray(inputs["x"], np.float32)
    assert x.shape == (B, T, J, C)
    ob2 = np.asarray(inputs["ob2"], np.float32)
    in_maps = _make_in_maps(inputs)
    nc = _build_nc(NTOK_PAD, 1)
    res = run_bass_kernel_spmd(nc, in_maps, list(range(NCORES)))
    global LAST_RESULT
    LAST_RESULT = res
    outs = [_unshard_out(res.results[i]["out"]) for i in range(NCORES)]
    full = np.concatenate(outs, axis=0)
    full = full + ob2.reshape(1, 1, 1, D)
    return full.astype(np.float32)


# revision 12
# speedup vs baseline: 1.9618x; 1.0919x over previous
"""Trainium2 Bass kernel for EnhancedMotionFlow (uniform f32r / K=128 version).

Pure data-parallel: batch dim B=256 sharded 32-per-core across 8 cores;
tiny weights replicated. No collectives.

Math (per token t = (b, T, J)):
  orig   = relu(x @ ow1 + ob1) @ ow2 + ob2                    x: 3 -> 64
  for each scale s in (1, 8, 25, 100):
    m_s  = depthwise temporal conv of x (clamped pad)         taps in {t-2..t+2}
    f_s  = relu(relu(m_s @ e_w1 + b1) @ e_w2 + b2) * softmax(sw)[s]
  combined = concat(f_s)                                      -> 64
  tmot  = relu(relu(combined @ tw1 + tb1) @ tw2 + tb2)        64 -> 128 -> 64
  out   = orig + tmot

Measured-on-silicon design notes (axon TRN2, repeat-regression wall clock):
  - f32r matmuls stream 512 cols in ~167 ns with FREE weight switches;
    fp8 plain is ~182 ns; fp8 DoubleRow and bf16 pay ~480 ns per weight
    switch. PE TILE-SIZE switches (K=8/16 vs K=128 modes) cost ~500-700 ns
    each -> every matmul here is padded to the K=128 tile mode and uses
    f32r, so the PE never changes mode or dtype (also makes the math exact).
  - engines can read only ONE PSUM operand per op and GPSIMD cannot read
    PSUM at all: orig (F) and tmot (E) are evicted to SBUF bf16 on Act/DVE
    and the final add runs on the otherwise-idle Pool engine.
  - eviction work is column-split between Act and DVE to balance busy time.
  - output stored feature-major bf16, one DMA per 2 pairs; the host
    reassembles token order, upcasts, and adds ob2.

Device-side structure (tokens on the free axis, chunk = 512 tokens,
pair = 2 chunks; per pair 10 matmuls / 7 eviction ops):
  xsf [16, NTOK] f32r: row 0 = ones (bias row), rows 1..15 = (tap, channel)
    pre-shifted + edge-clamped token streams (conv folded into L1 weights);
    zero-padded to 128 SBUF partitions on device.
  xf8 [8, NTOK/2] f32r: pair-folded ones + center-tap channels (chunk i
    rows 0:4, chunk j rows 4:8) for the orig path.
  A_i/A_j: h1 = relu(w1c^T x)            [128, 512] x2   (K=16 in 128 pad)
  B:       hop = relu(o1^T x) packed     [128, 512]      (K=8 in 128 pad)
  C1+C2:   comb = relu(w2^T h1 + b2c)    [128, 512]      (K=128 x2 acc)
  D_i/D_j: h2 = relu(tw1^T comb + tb1)   [128, 512] x2   (K=128)
  E1+E2:   tmot = relu(tw2^T h2 + tb2)   [128, 512]      (K=128 x2 acc)
  F:       forig = o2bd^T hop            [128, 512]      (K=128)
  fm = forig + tmot on Pool -> bf16 -> HBM.
"""

import sys
import functools

import numpy as np
import ml_dtypes

sys.path.insert(0, "/opt/trn_rl_repo")

from contextlib import ExitStack  # noqa: E402

import concourse.bacc as bacc  # noqa: E402
import concourse.tile as tile  # noqa: E402
from concourse import mybir  # noqa: E402
from concourse.bass_utils import run_bass_kernel_spmd  # noqa: E402

F32 = mybir.dt.float32
F32R = mybir.dt.float32r
BF16 = mybir.dt.bfloat16
Relu = mybir.ActivationFunctionType.Relu
ADD = mybir.AluOpType.add
MAX = mybir.AluOpType.max
NPB = ml_dtypes.bfloat16

B, T, J, C = 256, 243, 17, 3
D = 64
SD = 16
SCALES = (1, 8, 25, 100)
NCORES = 8
BLOC = B // NCORES            # batches per core
NTOK_FULL = BLOC * T * J      # tokens per core = 132192

CH = 512                      # tokens per chunk = one full PSUM bank of fp32
PAIR = 2 * CH                 # tokens per chunk-pair = 1024
NTOK_PAD = -(-NTOK_FULL // (4 * PAIR)) * (4 * PAIR)  # 135168 (33 x 4 pairs)

TAP_ORDER = (0, -2, -1, +1, +2)

# column split of the hop eviction: first HOP_ACT cols on Act, rest on DVE
HOP_ACT = 192


# ---------------------------------------------------------------------------
# host-side preprocessing
# ---------------------------------------------------------------------------

def _host_prep_x(x_shard: np.ndarray, ntok: int):
    """x_shard [BLOC', T, J, C] -> (xsf [16, ntok] f32, xf8 [8, ntok/2] f32).

    xsf: row 0 all-ones; row 1 + 3*g + c is channel c shifted by TAP_ORDER[g]
    t-steps with clamped padding. xf8 = rows 0:4 pair-folded (chunk i rows
    0:4, chunk j rows 4:8). Token order: tok = b*T*J + t*J + j.
    """
    bl = x_shard.shape[0]
    full = bl * T * J
    n = min(full, ntok)
    xs = np.zeros((16, ntok), dtype=np.float32)
    xs[0, :] = 1.0
    tidx = np.arange(T)
    for g, off in enumerate(TAP_ORDER):
        src_t = np.clip(tidx + off, 0, T - 1)
        shifted = x_shard[:, src_t, :, :]             # [bl, T, J, C]
        for c in range(C):
            xs[1 + 3 * g + c, :n] = shifted[:, :, :, c].reshape(full)[:n]
    f = xs[0:4].reshape(4, ntok // PAIR, 2, CH)
    xf8 = np.concatenate([f[:, :, 0, :], f[:, :, 1, :]], axis=0)
    return xs, np.ascontiguousarray(xf8.reshape(8, ntok // 2))


def _host_prep_weights(p: dict) -> dict:
    """Fold the tiny parameters into device weight tiles (all f32, K-padded
    to 128 rows so every matmul runs in the same PE tile mode)."""
    w = {}
    sw = np.asarray(p["sw"], np.float32)
    e = np.exp(sw - sw.max())
    wts = (e / e.sum()).astype(np.float32)

    # w1c: fused conv taps x encoder L1, L1 bias in the ones row.
    w1c = np.zeros((128, 128), np.float32)
    for i, s in enumerate(SCALES):
        ks = min(s + 1, 5)
        kern = np.asarray(p[f"k{s}"], np.float32)      # [ks, C]
        w1 = np.asarray(p[f"e{s}w1"], np.float32)      # [C, 32]
        b1 = np.asarray(p[f"e{s}b1"], np.float32)      # [32]
        for k in range(ks):
            off = k - ks // 2
            g = TAP_ORDER.index(off)
            for c in range(C):
                w1c[1 + 3 * g + c, 32 * i:32 * i + 32] += kern[k, c] * w1[c, :]
        w1c[0, 32 * i:32 * i + 32] = b1
    w["aw"] = w1c

    # orig embed L1 as one K=8 (128-pad) matmul over the pair-folded xf8:
    # rows 0:4 (chunk i) -> out cols 0:64, rows 4:8 (chunk j) -> cols 64:128.
    o1 = np.zeros((4, D), np.float32)
    o1[0, :] = np.asarray(p["ob1"], np.float32)
    o1[1:4, :] = np.asarray(p["ow1"], np.float32)
    bw = np.zeros((128, 128), np.float32)
    bw[0:4, 0:64] = o1
    bw[4:8, 64:128] = o1
    w["bw"] = bw

    # encoder L2 block-diagonal with softmax weights folded; two K=128
    # accumulation halves (cwa: h1_i -> comb cols 0:64 / cwb: h1_j -> 64:128).
    w2 = np.zeros((128, D), np.float32)
    b2c = np.zeros((D,), np.float32)
    for i, s in enumerate(SCALES):
        w2[32 * i:32 * i + 32, 16 * i:16 * i + 16] = wts[i] * np.asarray(
            p[f"e{s}w2"], np.float32)
        b2c[16 * i:16 * i + 16] = wts[i] * np.asarray(p[f"e{s}b2"], np.float32)
    cwa = np.zeros((128, 128), np.float32)
    cwa[:, 0:64] = w2
    cwb = np.zeros((128, 128), np.float32)
    cwb[:, 64:128] = w2
    w["cwa"], w["cwb"] = cwa, cwb
    w["b2x2"] = np.concatenate([b2c, b2c]).reshape(128, 1).astype(np.float32)

    # transform L1: dwa acts on comb rows 0:64 (comb_i), dwb on rows 64:128.
    tw1 = np.asarray(p["tw1"], np.float32)             # [64, 128]
    dwa = np.zeros((128, 128), np.float32)
    dwa[0:64, :] = tw1
    dwb = np.zeros((128, 128), np.float32)
    dwb[64:128, :] = tw1
    w["dwa"], w["dwb"] = dwa, dwb
    w["tb1"] = np.asarray(p["tb1"], np.float32).reshape(128, 1)

    # transform L2 accumulation halves.
    tw2 = np.asarray(p["tw2"], np.float32)             # [128, 64]
    ewa = np.zeros((128, 128), np.float32)
    ewa[:, 0:64] = tw2
    ewb = np.zeros((128, 128), np.float32)
    ewb[:, 64:128] = tw2
    w["ewa"], w["ewb"] = ewa, ewb
    tb2 = np.asarray(p["tb2"], np.float32)
    w["tb2x2"] = np.concatenate([tb2, tb2]).reshape(128, 1).astype(np.float32)

    # orig embed L2 block-diag.
    ow2 = np.asarray(p["ow2"], np.float32)             # [64, 64]
    o2bd = np.zeros((128, 128), np.float32)
    o2bd[:64, :64] = ow2
    o2bd[64:, 64:] = ow2
    w["fw"] = o2bd
    return w


# name -> (shape, dtype kind); "r" = f32r (f32 bytes)
WEIGHT_SHAPES = {
    "aw": ((128, 128), "r"), "bw": ((128, 128), "r"),
    "cwa": ((128, 128), "r"), "cwb": ((128, 128), "r"),
    "b2x2": ((128, 1), "f"),
    "dwa": ((128, 128), "r"), "dwb": ((128, 128), "r"),
    "tb1": ((128, 1), "f"),
    "ewa": ((128, 128), "r"), "ewb": ((128, 128), "r"),
    "tb2x2": ((128, 1), "f"),
    "fw": ((128, 128), "r"),
}
_DT = {"b": BF16, "f": F32, "r": F32R}
_NPDT = {"b": NPB, "f": np.float32, "r": np.float32}


# ---------------------------------------------------------------------------
# device kernel
# ---------------------------------------------------------------------------

def _emit(ctx: ExitStack, tc: tile.TileContext, ntok: int,
          xs_d, xf_d, w_d, out_d, repeat: int = 1, variant: str = "full"):
    nc = tc.nc
    npairs = ntok // PAIR
    assert npairs * PAIR == ntok and npairs % 4 == 0

    singles = ctx.enter_context(tc.tile_pool(name="singles", bufs=1))
    work = ctx.enter_context(tc.tile_pool(name="work", bufs=2))
    outp = ctx.enter_context(tc.tile_pool(name="outs", bufs=2))
    psum = ctx.enter_context(tc.tile_pool(name="psum", bufs=1, space="PSUM"))

    do_mm = "nomm" not in variant
    do_ev = "noev" not in variant

    wt = {}
    for name, (shape, kind) in WEIGHT_SHAPES.items():
        t = singles.tile(list(shape), _DT[kind], tag=f"w_{name}")
        nc.sync.dma_start(out=t[:], in_=w_d[name][:])
        wt[name] = t

    if not (do_mm and do_ev):
        statics = {}
        for nm, sh, dt in (("sh1", [128, 1024], F32R),
                           ("shop", [128, 512], F32R),
                           ("scomb", [128, 512], F32R),
                           ("sh2", [128, 1024], F32R),
                           ("stmot", [128, 512], BF16),
                           ("sforig", [128, 512], BF16),
                           ("sfm", [128, 1024], BF16)):
            t = singles.tile(sh, dt, tag=nm)
            nc.vector.memset(t[:].bitcast(F32) if dt == F32R else t[:], 0)
            statics[nm] = t

    # x rings: 2 slots of 4 pairs each; rows above the real data stay zero
    # so every matmul contracts a full K=128 without PE tile-mode switches.
    xring = singles.tile([128, 2 * 4 * PAIR], F32R, tag="xring")
    nc.vector.memset(xring[:].bitcast(F32), 0.0)
    fring = singles.tile([128, 2 * 4 * CH], F32R, tag="fring")
    nc.vector.memset(fring[:].bitcast(F32), 0.0)

    for pi_r in range(npairs * repeat):
        pi = pi_r % npairs
        slot = (pi // 4) % 2
        if pi % 4 == 0:
            nc.sync.dma_start(
                out=xring[0:16, slot * 4 * PAIR:(slot + 1) * 4 * PAIR],
                in_=xs_d[:, pi * PAIR:(pi + 4) * PAIR])
            nc.sync.dma_start(
                out=fring[0:8, slot * 4 * CH:(slot + 1) * 4 * CH],
                in_=xf_d[:, pi * CH:(pi + 4) * CH])
        xo = slot * 4 * PAIR + (pi % 4) * PAIR
        x_i = xring[:, xo:xo + CH]
        x_j = xring[:, xo + CH:xo + PAIR]
        fo = slot * 4 * CH + (pi % 4) * CH
        f_p = fring[:, fo:fo + CH]

        # ---- branch L1 (conv + encoder L1 fused): h1 = relu(w1c^T x)
        if do_mm:
            A = psum.tile([128, 1024], F32, tag="A")
            nc.tensor.matmul(A[:, 0:512], wt["aw"][:], x_i, start=True,
                             stop=True)
            nc.tensor.matmul(A[:, 512:1024], wt["aw"][:], x_j, start=True,
                             stop=True)
        if do_mm and do_ev:
            h1 = work.tile([128, 1024], F32R, tag="h1", bufs=3)
            nc.scalar.activation(h1[:], A[:], Relu)
        else:
            h1 = statics["sh1"]

        # ---- orig embed L1: hop = relu(o1^T x) packed, K=8 pair-folded
        if do_mm:
            Bp = psum.tile([128, 512], F32, tag="B")
            nc.tensor.matmul(Bp[:], wt["bw"][:], f_p, start=True, stop=True)
        if do_mm and do_ev:
            hop = work.tile([128, 512], F32R, tag="hop", bufs=3)
            nc.scalar.activation(hop[:, 0:HOP_ACT], Bp[:, 0:HOP_ACT], Relu)
            nc.vector.tensor_scalar(hop[:, HOP_ACT:512], Bp[:, HOP_ACT:512],
                                    0.0, None, op0=MAX)
        else:
            hop = statics["shop"]

        # ---- encoder L2: comb = relu(w2^T h1 + b2c), two-K accumulation
        if do_mm:
            Cp = psum.tile([128, 512], F32, tag="C")
            nc.tensor.matmul(Cp[:], wt["cwa"][:], h1[:, 0:512], start=True,
                             stop=False)
            nc.tensor.matmul(Cp[:], wt["cwb"][:], h1[:, 512:1024], start=False,
                             stop=True)
        if do_mm and do_ev:
            comb = work.tile([128, 512], F32R, tag="comb", bufs=3)
            nc.vector.tensor_scalar(comb[:], Cp[:], wt["b2x2"][:, 0:1], 0.0,
                                    op0=ADD, op1=MAX)
        else:
            comb = statics["scomb"]

        # ---- transform L1: h2 = relu(tw1^T comb + tb1)
        if do_mm:
            Dp = psum.tile([128, 1024], F32, tag="D")
            nc.tensor.matmul(Dp[:, 0:512], wt["dwa"][:], comb[:], start=True,
                             stop=True)
            nc.tensor.matmul(Dp[:, 512:1024], wt["dwb"][:], comb[:],
                             start=True, stop=True)
        if do_mm and do_ev:
            h2 = work.tile([128, 1024], F32R, tag="h2", bufs=3)
            nc.scalar.activation(h2[:], Dp[:], Relu, bias=wt["tb1"][:, 0:1])
        else:
            h2 = statics["sh2"]

        # ---- transform L2: tmot = relu(tw2^T h2 + tb2), two-K accumulation
        if do_mm:
            Ep = psum.tile([128, 512], F32, tag="E")
            nc.tensor.matmul(Ep[:], wt["ewa"][:], h2[:, 0:512], start=True,
                             stop=False)
            nc.tensor.matmul(Ep[:], wt["ewb"][:], h2[:, 512:1024], start=False,
                             stop=True)
        if do_mm and do_ev:
            tmot = work.tile([128, 512], BF16, tag="tmot", bufs=3)
            nc.vector.tensor_scalar(tmot[:], Ep[:], wt["tb2x2"][:, 0:1], 0.0,
                                    op0=ADD, op1=MAX)
        else:
            tmot = statics["stmot"]

        # ---- orig embed L2: forig = o2bd^T hop
        if do_mm:
            Fp = psum.tile([128, 512], F32, tag="F")
            nc.tensor.matmul(Fp[:], wt["fw"][:], hop[:], start=True,
                             stop=True)
        if do_mm and do_ev:
            forig = work.tile([128, 512], BF16, tag="forig", bufs=3)
            nc.vector.tensor_scalar(forig[:], Fp[:], 0.0, None, op0=ADD)
        else:
            forig = statics["sforig"]

        # ---- final add on Pool (SBUF-only engine): fm = forig + tmot
        if do_mm and do_ev:
            if pi % 2 == 0:
                fm2 = outp.tile([128, 2 * 512], BF16, tag="fm2", bufs=2)
            fslice = fm2[:, (pi % 2) * 512:(pi % 2) * 512 + 512]
            nc.gpsimd.tensor_tensor(fslice, forig[:], tmot[:], op=ADD)
        else:
            fm2 = statics["sfm"]

        # ---- store 2 pairs per DMA, feature-major bf16
        if pi % 2 == 1:
            base = (pi - 1) * 128
            nc.sync.dma_start(
                out=out_d[base:base + 256, :].rearrange("(s p) n -> p s n",
                                                        p=128),
                in_=fm2[:].rearrange("p (s n) -> p s n", s=2))


@functools.lru_cache(maxsize=16)
def _build_nc(ntok: int, repeat: int = 1, variant: str = "full"):
    nc = bacc.Bacc("TRN2", target_bir_lowering=False, debug=False)
    xs_d = nc.dram_tensor("xsf", [16, ntok], F32R, kind="ExternalInput").ap()
    xf_d = nc.dram_tensor("xf8", [8, ntok // 2], F32R,
                          kind="ExternalInput").ap()
    w_d = {}
    for name, (shape, kind) in WEIGHT_SHAPES.items():
        w_d[name] = nc.dram_tensor(name, list(shape), _DT[kind],
                                   kind="ExternalInput").ap()
    npairs = ntok // PAIR
    out_d = nc.dram_tensor("out", [npairs * 128, 512], BF16,
                           kind="ExternalOutput").ap()
    with tile.TileContext(nc) as tc:
        with ExitStack() as ctx:
            _emit(ctx, tc, ntok, xs_d, xf_d, w_d, out_d, repeat=repeat,
                  variant=variant)
    nc.compile()
    return nc


# ---------------------------------------------------------------------------
# entry point
# ---------------------------------------------------------------------------

LAST_RESULT = None


def _unshard_out(raw: np.ndarray) -> np.ndarray:
    """[npairs*128, 512] bf16 -> [BLOC, T, J, D] f32 token-major."""
    npairs = NTOK_PAD // PAIR
    a = raw.astype(np.float32).reshape(npairs, 2, D, CH)  # [p, ch, feat, n]
    a = a.transpose(0, 1, 3, 2).reshape(npairs * PAIR, D)
    return a[:NTOK_FULL].reshape(BLOC, T, J, D)


def _make_in_maps(inputs) -> list:
    x = np.asarray(inputs["x"], np.float32)
    w = _host_prep_weights(inputs)
    in_maps = []
    for ci in range(NCORES):
        xsf, xf8 = _host_prep_x(x[ci * BLOC:(ci + 1) * BLOC], NTOK_PAD)
        m = {"xsf": xsf, "xf8": xf8}
        for name in WEIGHT_SHAPES:
            m[name] = w[name]
        in_maps.append(m)
    return in_maps


def kernel(**inputs) -> np.ndarray:
    x = np.asarray(inputs["x"], np.float32)
    assert x.shape == (B, T, J, C)
    ob2 = np.asarray(inputs["ob2"], np.float32)
    in_maps = _make_in_maps(inputs)
    nc = _build_nc(NTOK_PAD, 1)
    res = run_bass_kernel_spmd(nc, in_maps, list(range(NCORES)))
    global LAST_RESULT
    LAST_RESULT = res
    outs = [_unshard_out(res.results[i]["out"]) for i in range(NCORES)]
    full = np.concatenate(outs, axis=0)
    full = full + ob2.reshape(1, 1, 1, D)
    return full.astype(np.float32)
